# revision 16
# baseline (speedup 1.0000x reference)
"""DPINet GNN message passing on 8 Trainium2 NeuronCores (Bass/Tile).

Strategy (receiver-partitioned, dense node updates):
  - Core c owns node rows [6250c, 6250(c+1)); edges assigned to the core
    owning their receiver, sorted by receiver, tiled into 49 node-tiles of
    128 consecutive local nodes with a static per-tile edge capacity
    (max over cores, so all 8 cores share one NEFF).
  - Per prop step: edge MLPs run in transposed [feat, edge] layout on the
    tensor engine; the segment-sum over receivers is a matmul with an
    on-device-generated one-hot selection matrix; node updates are dense
    (masked), so the only true gather is the sender-feature (es) gather,
    done with large indirect DMAs from a replicated node_effects table that
    is refreshed between steps by an AllGather collective.
  - The final output only depends on per-instance means of node_effects, so
    each core returns its [6250, 128] slice and the tiny remaining math
    (node_out MLP, rot6d, rigid transform) runs on host.
"""
import sys
sys.path.insert(0, '/opt/trn_rl_repo')

import numpy as np

N_NODES = 50000
N_EDGES = 400000
NODE_DIM = 6
NODE_ATTR_DIM = 9
REL_ATTR_DIM = 4
HID = 128
N_STAGES = 2
N_INST = 10
ROT_OUT = 9
POSVEL_STD = np.array([1, 1, 1, 0.1, 0.1, 0.1], dtype=np.float32)

N_CORES = 8
SLICE = N_NODES // N_CORES          # 6250
NL = 6272                           # padded local nodes (49 * 128)
NT = NL // 128                      # 49 node tiles
SPAN = 512                          # max edge columns per L1 psum
GCH = 2048                          # es gather chunk columns
PROP_STEPS = 2
DEBUG_STEPS = False
DBG_TILE = 0

# weight stack indices
W_NE1, W_NE2, W_NE3 = 0, 1, 2
W_RE1, W_RE2, W_RE3 = 3, 4, 5
W_P_ER, W_P_ES, W_P_RC, W_P2 = 6, 7, 8, 9
W_N_NE, W_N_ER, W_N_AG, W_N2 = 10, 11, 12, 13
NW = 14
# bias column indices
B_NE1, B_NE2, B_NE3, B_RE1, B_RE2, B_RE3, B_P1, B_N1, B_P2, IOTA_COL = range(10)
NB = 10


# ---------------------------------------------------------------- host prep

def _mlp_np(ps, x):
    for i, (W, b) in enumerate(ps):
        x = x @ W + b
        if i < len(ps) - 1:
            x = np.maximum(x, 0.0)
    return x


def _prep(nodes, node_attrs, rels, rel_attrs, rel_stages, instance_idx, params):
    """Host preprocessing -> (per-core in_maps, static cfg, host context)."""
    std = POSVEL_STD
    nodes_n = (nodes / std).astype(np.float32)
    inst = np.asarray(instance_idx).astype(np.int64)
    n_inst = len(inst) - 1
    seg = np.searchsorted(inst, np.arange(N_NODES), side='right') - 1
    counts = np.diff(inst).astype(np.float32)
    sums = np.zeros((n_inst, NODE_DIM), np.float32)
    np.add.at(sums, seg, nodes_n)
    inst_mean = sums / counts[:, None]
    offsets = nodes_n - inst_mean[seg]
    attrs = np.concatenate([node_attrs.astype(np.float32), offsets], axis=1)  # [N,15]
    xfeat = np.concatenate([nodes_n, attrs], axis=1)  # [N,21]

    rs = np.asarray(rel_stages)
    rnp = np.asarray(rels)
    ra_all = np.asarray(rel_attrs, np.float32)

    cfg = {'L': {}, 'EP': {}}
    in_maps = [dict() for _ in range(N_CORES)]

    for c in range(N_CORES):
        xn = np.zeros((21, NL), np.float32)
        xn[:, :SLICE] = xfeat[c * SLICE:(c + 1) * SLICE].T
        for s in range(N_STAGES):
            in_maps[c][f'xnode{s}'] = xn

    for s in range(N_STAGES):
        eidx = np.nonzero(rs == s)[0]
        r_all = rnp[eidx, 0]
        s_all = rnp[eidx, 1]
        ra_s = ra_all[eidx]
        owner = r_all // SLICE

        per_core = []
        tile_counts = np.zeros((N_CORES, NT), np.int64)
        for c in range(N_CORES):
            m = owner == c
            r_c, s_c, ra_c = r_all[m], s_all[m], ra_s[m]
            order = np.argsort(r_c, kind='stable')
            r_c, s_c, ra_c = r_c[order], s_c[order], ra_c[order]
            rid = (r_c - c * SLICE).astype(np.int64)
            tiles = rid // 128
            np.add.at(tile_counts[c], tiles, 1)
            per_core.append((r_c, s_c, ra_c, rid, tiles))

        L = np.maximum(128, ((tile_counts.max(axis=0) + 127) // 128) * 128)
        A = np.concatenate([[0], np.cumsum(L)])
        EP = int(A[-1])
        cfg['L'][s] = [int(x) for x in L]
        cfg['EP'][s] = EP

        for c in range(N_CORES):
            r_c, s_c, ra_c, rid, tiles = per_core[c]
            pos = np.zeros(len(r_c), np.int64)
            # position of each edge: A[tile] + index within tile
            off = np.zeros(NT, np.int64)
            for t in range(NT):
                m = tiles == t
                k = int(m.sum())
                pos[m] = A[t] + np.arange(k)
                off[t] = k

            xrel = np.zeros((46, EP), np.float32)
            xrel[0:6, pos] = nodes_n[r_c].T
            xrel[6:21, pos] = attrs[r_c].T
            xrel[21:27, pos] = nodes_n[s_c].T
            xrel[27:42, pos] = attrs[s_c].T
            xrel[42:46, pos] = ra_c.T

            ridr = np.full((1, EP), -1e6, np.float32)
            ridr[0, pos] = rid.astype(np.float32)
            ridc = ridr.reshape(EP // 128, 128).T.copy()  # [128, EP/128]

            esg = np.zeros(EP, np.int32)
            esg[pos] = s_c.astype(np.int32)
            esidx = esg.reshape(EP // 128, 128).T.copy()  # [128, EP/128]
            # two-table int16 indices into the AG'd ext table [50008, 128]
            row = 6251 * (esg // SLICE) + (esg % SLICE)
            lo = row <= 32767
            idx_lo = np.where(lo, row, 6250).astype(np.int16)
            idx_hi = np.where(lo, 32767, row - 17240).astype(np.int16)

            def wrap16(a):
                w = a.reshape(EP // 16, 16).T
                return np.tile(w, (8, 1)).copy()  # [128, EP/16]

            cnt = np.zeros(NL, np.float32)
            np.add.at(cnt, rid, 1.0)
            mask = (cnt > 0).astype(np.float32)
            bp2 = np.asarray(params['rel_prop'][s][1][1], np.float32)  # [128]

            im = in_maps[c]
            im[f'xrel{s}'] = xrel
            im[f'ridr{s}'] = ridr
            im[f'ridc{s}'] = ridc
            im[f'esidx{s}'] = esidx
            im[f'eslo{s}'] = wrap16(idx_lo)
            im[f'eshi{s}'] = wrap16(idx_hi)
            im[f'aggc{s}'] = bp2[:, None] * cnt[None, :]   # [128, NL]
            im[f'mask{s}'] = mask.reshape(NT, 128).T.copy()  # [128, NT]

    # weights / biases (shared across cores)
    for s in range(N_STAGES):
        wk = np.zeros((NW, 128, 128), np.float32)
        bias = np.zeros((128, NB), np.float32)

        ne = [(np.asarray(W), np.asarray(b)) for W, b in params['node_enc'][s]]
        re = [(np.asarray(W), np.asarray(b)) for W, b in params['rel_enc'][s]]
        rp = [(np.asarray(W), np.asarray(b)) for W, b in params['rel_prop'][s]]
        npr = [(np.asarray(W), np.asarray(b)) for W, b in params['node_prop'][s]]

        wk[W_NE1, :21] = ne[0][0]
        wk[W_NE2], wk[W_NE3] = ne[1][0], ne[2][0]
        wk[W_RE1, :46] = re[0][0]
        wk[W_RE2], wk[W_RE3] = re[1][0], re[2][0]
        wk[W_P_ER] = rp[0][0][0:128]
        wk[W_P_ES] = rp[0][0][128:256]
        wk[W_P_RC] = rp[0][0][256:384]
        wk[W_P2] = rp[1][0]
        wk[W_N_NE] = npr[0][0][0:128]
        wk[W_N_ER] = npr[0][0][128:256]
        wk[W_N_AG] = npr[0][0][256:384]
        wk[W_N2] = npr[1][0]

        bias[:, B_NE1], bias[:, B_NE2], bias[:, B_NE3] = ne[0][1], ne[1][1], ne[2][1]
        bias[:, B_RE1], bias[:, B_RE2], bias[:, B_RE3] = re[0][1], re[1][1], re[2][1]
        bias[:, B_P1] = rp[0][1]
        bias[:, B_N1] = npr[0][1]
        bias[:, B_P2] = rp[1][1]
        bias[:, IOTA_COL] = np.arange(128, dtype=np.float32)
        brow = npr[1][1].reshape(1, 128).astype(np.float32)  # node_prop L2 bias

        for c in range(N_CORES):
            im = in_maps[c]
            im[f'w{s}'] = wk
            im[f'bias{s}'] = bias
            im[f'brow{s}'] = brow

    iotar = np.broadcast_to(np.arange(128, dtype=np.float32), (128, 128)).copy()
    for c in range(N_CORES):
        in_maps[c]['iotar'] = iotar

    host = dict(seg=seg, counts=counts, nodes_n=nodes_n, n_inst=n_inst)
    return in_maps, cfg, host


# ------------------------------------------------------- numpy device model

def np_device_model(in_maps, cfg, debug=False):
    """Numpy replica of the device program (same tiling), for validation.
    Returns list of per-core ne_out [SLICE, HID] (and debug snaps if asked)."""
    NE_full = np.zeros((N_NODES, HID), np.float32)
    er_store = [np.zeros((NL, HID), np.float32) for _ in range(N_CORES)]
    snaps = {}

    def relu(x):
        return np.maximum(x, 0.0)

    first = True
    for s in range(N_STAGES):
        L = cfg['L'][s]
        A = np.concatenate([[0], np.cumsum(L)]).astype(int)
        EP = cfg['EP'][s]
        # per-core stage constants
        renc, neT = [], []
        for c in range(N_CORES):
            im = in_maps[c]
            wk, bias = im[f'w{s}'], im[f'bias{s}']
            x = im[f'xrel{s}']  # [46, EP]
            h = relu(wk[W_RE1, :46].T @ x + bias[:, B_RE1:B_RE1 + 1])
            h = relu(wk[W_RE2].T @ h + bias[:, B_RE2:B_RE2 + 1])
            renc.append(wk[W_RE3].T @ h + bias[:, B_RE3:B_RE3 + 1])  # [128, EP]
            xn = im[f'xnode{s}']
            h = relu(wk[W_NE1, :21].T @ xn + bias[:, B_NE1:B_NE1 + 1])
            h = relu(wk[W_NE2].T @ h + bias[:, B_NE2:B_NE2 + 1])
            neT.append(wk[W_NE3].T @ h + bias[:, B_NE3:B_NE3 + 1])  # [128, NL]
        if debug and s == 0:
            snaps['renc0'] = [r.copy() for r in renc]
            snaps['neT0'] = [n.copy() for n in neT]

        for k in range(PROP_STEPS):
            new_slices = []
            for c in range(N_CORES):
                im = in_maps[c]
                wk, bias, brow = im[f'w{s}'], im[f'bias{s}'], im[f'brow{s}']
                ridr = im[f'ridr{s}'][0]  # [EP]
                esg = im[f'esidx{s}'].T.reshape(-1)  # [EP] global sender ids
                aggc = im[f'aggc{s}']  # [128, NL]
                mask = im[f'mask{s}'].T.reshape(-1)
                es_T = np.zeros((HID, EP), np.float32) if first else NE_full[esg].T
                er = er_store[c]

                for t in range(NT):
                    a, b = A[t], A[t + 1]
                    erT_t = er[t * 128:(t + 1) * 128].T  # [feat, 128]
                    # L1 over spans
                    h1 = np.zeros((HID, b - a), np.float32)
                    stt = (ridr[a:b, None] == (np.arange(128) + 128 * t)[None, :])
                    stt = stt.astype(np.float32).T  # [128 r, span_total]
                    x1 = wk[W_P_RC].T @ renc[c][:, a:b]
                    if not first:
                        x1 = x1 + wk[W_P_ES].T @ es_T[:, a:b]
                        gt = er[t * 128:(t + 1) * 128] @ wk[W_P_ER]  # [r, out]
                        x1 = x1 + gt.T @ stt
                    h1 = relu(x1 + bias[:, B_P1:B_P1 + 1])
                    releff = h1.T @ wk[W_P2]  # [span, out], no bias (folded)
                    # segment sum via S
                    Smat = stt.T  # [e, r]
                    agg = releff.T @ Smat  # [feat(out) x r]? careful:
                    # agg[f, r] = sum_e releff[e, f] * S[e, r]
                    agg = np.einsum('ef,er->fr', releff, Smat)
                    agg = agg + aggc[:, t * 128:(t + 1) * 128]
                    # node prop
                    xnp_ = wk[W_N_NE].T @ neT[c][:, t * 128:(t + 1) * 128] \
                        + wk[W_N_AG].T @ agg
                    if not first:
                        xnp_ = xnp_ + wk[W_N_ER].T @ erT_t
                    hnp = relu(xnp_ + bias[:, B_N1:B_N1 + 1])
                    upd = hnp.T @ wk[W_N2] + brow  # [r, feat]
                    m = mask[t * 128:(t + 1) * 128][:, None]
                    er[t * 128:(t + 1) * 128] = upd * m + er[t * 128:(t + 1) * 128]
                new_slices.append(er[:SLICE].copy())
            if debug:
                snaps[(s, k)] = [er_store[c].copy() for c in range(N_CORES)]
            NE_full = np.concatenate(new_slices, axis=0)
            first = False
    if debug:
        return [er_store[c][:SLICE] for c in range(N_CORES)], snaps
    return [er_store[c][:SLICE] for c in range(N_CORES)]


# ------------------------------------------------------------- bass program

def build_nc(cfg):
    from concourse import bass, bacc, mybir, tile
    f32 = mybir.dt.float32
    i32 = mybir.dt.int32
    AF = mybir.ActivationFunctionType
    OP = mybir.AluOpType

    nc = bacc.Bacc("TRN2", target_bir_lowering=False, debug=False,
                   num_devices=N_CORES)

    # --- I/O declarations
    ins = {}
    for s in range(N_STAGES):
        EP = cfg['EP'][s]
        ins[f'xnode{s}'] = nc.dram_tensor(f'xnode{s}', [21, NL], f32, kind="ExternalInput")
        ins[f'xrel{s}'] = nc.dram_tensor(f'xrel{s}', [46, EP], f32, kind="ExternalInput")
        ins[f'ridc{s}'] = nc.dram_tensor(f'ridc{s}', [128, EP // 128], f32, kind="ExternalInput")
        ins[f'eslo{s}'] = nc.dram_tensor(f'eslo{s}', [128, EP // 16], mybir.dt.int16, kind="ExternalInput")
        ins[f'eshi{s}'] = nc.dram_tensor(f'eshi{s}', [128, EP // 16], mybir.dt.int16, kind="ExternalInput")
        ins[f'aggc{s}'] = nc.dram_tensor(f'aggc{s}', [128, NL], f32, kind="ExternalInput")
        ins[f'mask{s}'] = nc.dram_tensor(f'mask{s}', [128, NT], f32, kind="ExternalInput")
        ins[f'w{s}'] = nc.dram_tensor(f'w{s}', [NW, 128, 128], f32, kind="ExternalInput")
        ins[f'bias{s}'] = nc.dram_tensor(f'bias{s}', [128, NB], f32, kind="ExternalInput")
        ins[f'brow{s}'] = nc.dram_tensor(f'brow{s}', [1, 128], f32, kind="ExternalInput")
    ins['iotar'] = nc.dram_tensor('iotar', [128, 128], f32, kind="ExternalInput")
    ne_out = nc.dram_tensor('ne_out', [NL, HID], f32, kind="ExternalOutput")
    dbg_outs = []
    if DEBUG_STEPS:
        for i in range(N_STAGES * PROP_STEPS):
            dbg_outs.append(nc.dram_tensor(f'dbg{i}', [128, NL], f32,
                                           kind="ExternalOutput"))
        dbg_renc = nc.dram_tensor('dbgrenc', [128, 8192], f32, kind="ExternalOutput")
        dbg_ne = nc.dram_tensor('dbgne', [128, NL], f32, kind="ExternalOutput")
        dbg_t = {nm: nc.dram_tensor(f'dbg_{nm}', [128, sz], f32, kind="ExternalOutput")
                 for nm, sz in [('esraw', GCH), ('esT', 1536), ('erT', 128),
                                ('gt', 128), ('stt', SPAN), ('agg', 128),
                                ('hnp', 128), ('Sm', 128), ('nef', 2048)]}

    with tile.TileContext(nc) as tc:
        with tc.tile_pool(name="dram", bufs=1, space="DRAM") as dram, \
             tc.tile_pool(name="pers", bufs=1) as pers, \
             tc.tile_pool(name="sb", bufs=2) as sb, \
             tc.tile_pool(name="ps", bufs=2, space="PSUM") as ps:

            # persistent SBUF
            er_store = pers.tile([128, NL], f32)       # er_store[p, 128t+f] = NE[128t+p, f]
            neT_sb = pers.tile([128, NL], f32)         # node_enc output (per stage, reused)
            iotar_sb = pers.tile([128, 128], f32)
            ident_sb = pers.tile([128, 128], f32)
            nc.sync.dma_start(out=iotar_sb[:], in_=ins['iotar'][:])
            from concourse.masks import make_identity
            make_identity(nc, ident_sb[:])
            nc.vector.memset(er_store[:], 0.0)

            # DRAM staging
            ne_slice = dram.tile([NL, HID], f32)
            NEXT = N_CORES * (SLICE + 1)  # 50008
            ne_fulls = [dram.tile([NEXT, HID], f32, addr_space="Shared",
                                  tag=f"nef{i}", name=f"nef{i}") for i in range(3)]
            EPmax = max(cfg['EP'].values())
            renc_dram = dram.tile([128, EPmax], f32)

            def stage_pre(s):
                """Load stage constants, compute node_enc + rel_enc."""
                EP = cfg['EP'][s]
                st = {}
                st['w'] = w_all = pers.tile([128, NW * 128], f32, tag="w", name="w_all")
                for i in range(NW):
                    nc.sync.dma_start(out=w_all[:, i * 128:(i + 1) * 128],
                                      in_=ins[f'w{s}'][i])
                st['bias'] = bias_sb = pers.tile([128, NB], f32, tag="bias", name="bias_sb")
                nc.sync.dma_start(out=bias_sb[:], in_=ins[f'bias{s}'][:])
                brow_sb = pers.tile([1, 128], f32, tag="brow", name="brow_sb")
                nc.sync.dma_start(out=brow_sb[:], in_=ins[f'brow{s}'][:])
                st['bn2rep'] = bn2rep = pers.tile([128, 128], f32, tag="bn2rep", name="bn2rep")
                nc.gpsimd.partition_broadcast(bn2rep[:], brow_sb[0:1, :])
                st['aggc'] = aggc_sb = pers.tile([128, NL], f32, tag="aggc", name="aggc_sb")
                nc.sync.dma_start(out=aggc_sb[:], in_=ins[f'aggc{s}'][:])
                st['mask'] = mask_sb = pers.tile([128, NT], f32, tag="mask", name="mask_sb")
                nc.sync.dma_start(out=mask_sb[:], in_=ins[f'mask{s}'][:])
                st['ridc'] = ridc_sb = pers.tile([128, EPmax // 128], f32, tag="ridc", name="ridc_sb")
                nc.sync.dma_start(out=ridc_sb[:, :EP // 128], in_=ins[f'ridc{s}'][:])
                st['eslo'] = eslo_sb = pers.tile([128, EPmax // 16], mybir.dt.int16, tag="eslo", name="eslo_sb")
                nc.sync.dma_start(out=eslo_sb[:, :EP // 16], in_=ins[f'eslo{s}'][:])
                st['eshi'] = eshi_sb = pers.tile([128, EPmax // 16], mybir.dt.int16, tag="eshi", name="eshi_sb")
                nc.sync.dma_start(out=eshi_sb[:, :EP // 16], in_=ins[f'eshi{s}'][:])

                def W(i):
                    return w_all[:, i * 128:(i + 1) * 128]

                def bcol(j):
                    return bias_sb[:, j:j + 1]
                st['W'], st['bcol'] = W, bcol

                # node_enc: xnode -> neT_sb
                xn_sb = pers.tile([21, NL], f32, tag="xn")
                nc.sync.dma_start(out=xn_sb[:], in_=ins[f'xnode{s}'][:])
                for a in range(0, NL, SPAN):
                    b = min(a + SPAN, NL)
                    p1 = ps.tile([128, SPAN], f32, tag="L1", bufs=2)
                    nc.tensor.matmul(p1[:, :b - a], lhsT=W(W_NE1)[:21, :],
                                     rhs=xn_sb[:21, a:b], start=True, stop=True)
                    h1 = sb.tile([128, SPAN], f32, tag="h1", bufs=2)
                    nc.scalar.activation(h1[:, :b - a], p1[:, :b - a], AF.Relu, bias=bcol(B_NE1))
                    p2 = ps.tile([128, SPAN], f32, tag="L1", bufs=2)
                    nc.tensor.matmul(p2[:, :b - a], lhsT=W(W_NE2), rhs=h1[:, :b - a],
                                     start=True, stop=True)
                    h2 = sb.tile([128, SPAN], f32, tag="h1", bufs=2)
                    nc.scalar.activation(h2[:, :b - a], p2[:, :b - a], AF.Relu, bias=bcol(B_NE2))
                    p3 = ps.tile([128, SPAN], f32, tag="L1", bufs=2)
                    nc.tensor.matmul(p3[:, :b - a], lhsT=W(W_NE3), rhs=h2[:, :b - a],
                                     start=True, stop=True)
                    nc.scalar.activation(neT_sb[:, a:b], p3[:, :b - a], AF.Identity, bias=bcol(B_NE3))

                # rel_enc: xrel (DRAM) -> renc_dram
                for a in range(0, EP, SPAN):
                    b = min(a + SPAN, EP)
                    xr = sb.tile([46, SPAN], f32, tag="xr", bufs=3)
                    nc.sync.dma_start(out=xr[:, :b - a], in_=ins[f'xrel{s}'][:, a:b])
                    p1 = ps.tile([128, SPAN], f32, tag="L1", bufs=2)
                    nc.tensor.matmul(p1[:, :b - a], lhsT=W(W_RE1)[:46, :],
                                     rhs=xr[:, :b - a], start=True, stop=True)
                    h1 = sb.tile([128, SPAN], f32, tag="h1", bufs=2)
                    nc.scalar.activation(h1[:, :b - a], p1[:, :b - a], AF.Relu, bias=bcol(B_RE1))
                    p2 = ps.tile([128, SPAN], f32, tag="L1", bufs=2)
                    nc.tensor.matmul(p2[:, :b - a], lhsT=W(W_RE2), rhs=h1[:, :b - a],
                                     start=True, stop=True)
                    h2 = sb.tile([128, SPAN], f32, tag="h1", bufs=2)
                    nc.scalar.activation(h2[:, :b - a], p2[:, :b - a], AF.Relu, bias=bcol(B_RE2))
                    p3 = ps.tile([128, SPAN], f32, tag="L1", bufs=2)
                    nc.tensor.matmul(p3[:, :b - a], lhsT=W(W_RE3), rhs=h2[:, :b - a],
                                     start=True, stop=True)
                    rt = sb.tile([128, SPAN], f32, tag="h1", bufs=2)
                    nc.scalar.activation(rt[:, :b - a], p3[:, :b - a], AF.Identity, bias=bcol(B_RE3))
                    nc.sync.dma_start(out=renc_dram[:, a:b], in_=rt[:, :b - a])
                return st

            def emit_step(s, st, first, ne_src, write_ne, last, dbg=None):
                """One propagation step.
                first: node_effects==0 (skip er/es paths)
                ne_src: DRAM tile to gather es from (None if first)
                write_ne: DRAM tile to write updated slice to (ne_slice or None)
                last: if True, also write ne_out."""
                EP = cfg['EP'][s]
                L = cfg['L'][s]
                A = [0]
                for t in range(NT):
                    A.append(A[-1] + L[t])
                W, bcol = st['W'], st['bcol']
                Lmax = max(L)

                # es gather (two int16 tables) + transpose into per-tile es_T
                es_lo, es_hi = {}, {}
                if not first:
                    for a in range(0, EP, GCH):
                        b = min(a + GCH, EP)
                        n = b - a
                        glo = sb.tile([128, GCH], f32, tag="eslo_g", bufs=3,
                                      name=f"glo{s}{a}")
                        ghi = sb.tile([128, GCH], f32, tag="eshi_g", bufs=3,
                                      name=f"ghi{s}{a}")
                        nc.gpsimd.dma_gather(
                            out_ap=glo[:, :n].rearrange("p (k f) -> p k f", f=128),
                            in_ap=ne_src[0:32768, :],
                            idxs_ap=st['eslo'][:, a // 16:b // 16],
                            num_idxs=n, num_idxs_reg=n, elem_size=128,
                            transpose=False, single_packet=False)
                        nc.gpsimd.dma_gather(
                            out_ap=ghi[:, :n].rearrange("p (k f) -> p k f", f=128),
                            in_ap=ne_src[17240:17240 + 32768, :],
                            idxs_ap=st['eshi'][:, a // 16:b // 16],
                            num_idxs=n, num_idxs_reg=n, elem_size=128,
                            transpose=False, single_packet=False)
                        es_lo[a], es_hi[a] = glo, ghi

                for t in range(NT):
                    at, bt = A[t], A[t + 1]
                    if not first:
                        # Phase A: er^T, G^T
                        ptr = ps.tile([128, 128], f32, tag="tr", bufs=2, name="ptr")
                        nc.tensor.transpose(ptr[:], er_store[:, t * 128:(t + 1) * 128],
                                            ident_sb[:])
                        erT = sb.tile([128, 128], f32, tag="erT", bufs=3, name="erT")
                        nc.vector.tensor_copy(erT[:], ptr[:])
                        pgt = ps.tile([128, 128], f32, tag="n", bufs=2, name="pgt")
                        nc.tensor.matmul(pgt[:], lhsT=erT[:], rhs=W(W_P_ER),
                                         start=True, stop=True)
                        gt = sb.tile([128, 128], f32, tag="gt", bufs=3, name="gt")
                        nc.vector.tensor_copy(gt[:], pgt[:])

                        # es^T for this tile
                        esT = sb.tile([128, Lmax], f32, tag="esT", bufs=2, name="esT")
                        for j in range(L[t] // 128):
                            gcol = at + j * 128
                            a0 = (gcol // GCH) * GCH
                            off = gcol % GCH
                            pt2 = ps.tile([128, 128], f32, tag="tr", bufs=2, name="pt2")
                            nc.tensor.matmul(pt2[:], lhsT=es_lo[a0][:, off:off + 128],
                                             rhs=ident_sb[:], is_transpose=True,
                                             start=True, stop=False)
                            nc.tensor.matmul(pt2[:], lhsT=es_hi[a0][:, off:off + 128],
                                             rhs=ident_sb[:], is_transpose=True,
                                             start=False, stop=True)
                            nc.vector.tensor_copy(esT[:, j * 128:(j + 1) * 128], pt2[:])
                        if dbg is not None and t == DBG_TILE:
                            nc.sync.dma_start(out=dbg['esT'][:, :L[t]], in_=esT[:, :L[t]])
                            nc.sync.dma_start(out=dbg['erT'][:], in_=erT[:])
                            nc.sync.dma_start(out=dbg['gt'][:], in_=gt[:])

                    # agg accumulator psum
                    pagg = ps.tile([128, 128], f32, tag="agg", bufs=1, name="pagg")

                    nspans = (L[t] + SPAN - 1) // SPAN
                    for si in range(nspans):
                        a = at + si * SPAN
                        b = min(at + L[t], a + SPAN)
                        ln = b - a
                        # S chunks (one-hot edge->receiver) + S^T via PE transpose
                        Sms = []
                        stt = None
                        if not first:
                            stt = sb.tile([128, SPAN], f32, tag="stt", bufs=2, name="stt")
                        for j in range(ln // 128):
                            Sm = sb.tile([128, 128], f32, tag="Sm", bufs=6, name="Sm")
                            col = (a + j * 128) // 128
                            nc.vector.tensor_scalar(
                                out=Sm[:], in0=iotar_sb[:],
                                scalar1=st['ridc'][:, col:col + 1],
                                scalar2=float(-128 * t),
                                op0=OP.subtract, op1=OP.is_equal)
                            if dbg is not None and t == DBG_TILE and si == 0 and j == 0:
                                nc.sync.dma_start(out=dbg['Sm'][:], in_=Sm[:])
                            Sms.append(Sm)
                            if not first:
                                ptS = ps.tile([128, 128], f32, tag="tr", bufs=2, name="ptS")
                                nc.tensor.transpose(ptS[:], Sm[:], ident_sb[:])
                                nc.vector.tensor_copy(stt[:, j * 128:(j + 1) * 128], ptS[:])
                        p1 = ps.tile([128, SPAN], f32, tag="L1", bufs=2, name="p1")
                        rc = sb.tile([128, SPAN], f32, tag="rc", bufs=3, name="rc")
                        nc.sync.dma_start(out=rc[:, :ln], in_=renc_dram[:, a:b])
                        nc.tensor.matmul(p1[:, :ln], lhsT=W(W_P_RC), rhs=rc[:, :ln],
                                         start=True, stop=first)
                        if not first:
                            nc.tensor.matmul(p1[:, :ln], lhsT=W(W_P_ES),
                                             rhs=esT[:, si * SPAN:si * SPAN + ln],
                                             start=False, stop=False)
                            if dbg is not None and t == DBG_TILE and si == 0:
                                nc.sync.dma_start(out=dbg['stt'][:, :ln], in_=stt[:, :ln])
                            nc.tensor.matmul(p1[:, :ln], lhsT=gt[:], rhs=stt[:, :ln],
                                             start=False, stop=True)
                        h1 = sb.tile([128, SPAN], f32, tag="h1e", bufs=2, name="h1e")
                        nc.scalar.activation(h1[:, :ln], p1[:, :ln], AF.Relu, bias=bcol(B_P1))

                        for j in range(ln // 128):
                            pe = ps.tile([128, 128], f32, tag="L2", bufs=2, name="pe")
                            nc.tensor.matmul(pe[:], lhsT=h1[:, j * 128:(j + 1) * 128],
                                             rhs=W(W_P2), start=True, stop=True)
                            re_sb = sb.tile([128, 128], f32, tag="re", bufs=2, name="re")
                            nc.vector.tensor_copy(re_sb[:], pe[:])
                            nc.tensor.matmul(pagg[:], lhsT=re_sb[:], rhs=Sms[j][:],
                                             start=(si == 0 and j == 0),
                                             stop=(si == nspans - 1 and j == ln // 128 - 1))

                    # Phase C
                    agg_sb = sb.tile([128, 128], f32, tag="aggs", bufs=2, name="aggs")
                    nc.vector.tensor_tensor(
                        out=agg_sb[:], in0=pagg[:],
                        in1=st['aggc'][:, t * 128:(t + 1) * 128], op=OP.add)

                    pnp = ps.tile([128, 128], f32, tag="n", bufs=2, name="pnp")
                    nc.tensor.matmul(pnp[:], lhsT=W(W_N_NE),
                                     rhs=neT_sb[:, t * 128:(t + 1) * 128],
                                     start=True, stop=first)
                    if not first:
                        nc.tensor.matmul(pnp[:], lhsT=W(W_N_ER), rhs=erT[:],
                                         start=False, stop=False)
                    nc.tensor.matmul(pnp[:], lhsT=W(W_N_AG), rhs=agg_sb[:],
                                     start=False, stop=True)
                    hnp = sb.tile([128, 128], f32, tag="hnp", bufs=2, name="hnp")
                    nc.scalar.activation(hnp[:], pnp[:], AF.Relu, bias=bcol(B_N1))
                    if dbg is not None and t == DBG_TILE:
                        nc.sync.dma_start(out=dbg['agg'][:], in_=agg_sb[:])
                        nc.sync.dma_start(out=dbg['hnp'][:], in_=hnp[:])
                    pup = ps.tile([128, 128], f32, tag="n", bufs=2, name="pup")
                    nc.tensor.matmul(pup[:], lhsT=hnp[:], rhs=W(W_N2), start=True, stop=True)

                    tmp = sb.tile([128, 128], f32, tag="tmp", bufs=2, name="tmp")
                    nc.vector.tensor_tensor(out=tmp[:], in0=pup[:],
                                            in1=st['bn2rep'][:], op=OP.add)
                    if first:
                        nc.vector.tensor_scalar(
                            out=er_store[:, t * 128:(t + 1) * 128], in0=tmp[:],
                            scalar1=st['mask'][:, t:t + 1], scalar2=None, op0=OP.mult)
                    else:
                        nc.vector.tensor_scalar(out=tmp[:], in0=tmp[:],
                                                scalar1=st['mask'][:, t:t + 1],
                                                scalar2=None, op0=OP.mult)
                        nc.vector.tensor_tensor(
                            out=er_store[:, t * 128:(t + 1) * 128], in0=tmp[:],
                            in1=er_store[:, t * 128:(t + 1) * 128], op=OP.add)

                er3 = er_store[:].rearrange("p (t f) -> p t f", f=128)
                if write_ne is not None:
                    nc.sync.dma_start(
                        out=write_ne[:].rearrange("(t p) f -> p t f", p=128),
                        in_=er3)
                if last:
                    nc.sync.dma_start(
                        out=ne_out[:].rearrange("(t p) f -> p t f", p=128),
                        in_=er3)

            # ---- full schedule
            agi = 0
            rg = [list(range(N_CORES))]
            from concourse import mybir as _mb
            for s in range(N_STAGES):
                st = stage_pre(s)
                if DEBUG_STEPS and s == 0:
                    nc.sync.dma_start(out=dbg_renc[:, :min(8192, cfg['EP'][0])],
                                      in_=renc_dram[:, :min(8192, cfg['EP'][0])])
                    nc.sync.dma_start(out=dbg_ne[:], in_=neT_sb[:])
                for k in range(PROP_STEPS):
                    first = (s == 0 and k == 0)
                    last = (s == N_STAGES - 1 and k == PROP_STEPS - 1)
                    ne_src = ne_fulls[agi - 1] if not first else None
                    dbg = dbg_t if (DEBUG_STEPS and s == 0 and k == 1) else None
                    emit_step(s, st, first, ne_src, None if last else ne_slice, last,
                              dbg=dbg)
                    if DEBUG_STEPS:
                        nc.sync.dma_start(out=dbg_outs[s * PROP_STEPS + k][:],
                                          in_=er_store[:])
                    if not last:
                        nc.gpsimd.collective_compute(
                            "AllGather", _mb.AluOpType.bypass, replica_groups=rg,
                            ins=[ne_slice[:SLICE + 1, :]], outs=[ne_fulls[agi][:]])
                        agi += 1

    nc.compile()
    return nc


# ------------------------------------------------------------------ runner

class _Runner:
    def __init__(self, nc, n_cores):
        import jax
        from jax.sharding import Mesh, PartitionSpec
        from jax.experimental.shard_map import shard_map
        from concourse import mybir
        from concourse.bass2jax import (_bass_exec_p, install_neuronx_cc_hook,
                                        partition_id_tensor)
        install_neuronx_cc_hook()
        self.jax = jax
        self.n_cores = n_cores
        partition_name = nc.partition_id_tensor.name if nc.partition_id_tensor else None
        in_names, out_names, out_avals, zero_outs = [], [], [], []
        for alloc in nc.m.functions[0].allocations:
            if not isinstance(alloc, mybir.MemoryLocationSet):
                continue
            name = alloc.memorylocations[0].name
            if alloc.kind == "ExternalInput":
                if name != partition_name:
                    in_names.append(name)
            elif alloc.kind == "ExternalOutput":
                out_names.append(name)
                shape = tuple(alloc.tensor_shape)
                dtype = mybir.dt.np(alloc.dtype)
                out_avals.append(jax.core.ShapedArray(shape, dtype))
                zero_outs.append(np.zeros(shape, dtype))
        self.in_names, self.out_names = in_names, out_names
        self.out_avals, self.zero_outs = out_avals, zero_outs
        n_params, n_outs = len(in_names), len(out_avals)
        all_in = list(in_names) + list(out_names)
        if partition_name is not None:
            all_in.append(partition_name)

        def _body(*args):
            operands = list(args)
            if partition_name is not None:
                operands.append(partition_id_tensor())
            outs = _bass_exec_p.bind(
                *operands, out_avals=tuple(out_avals), in_names=tuple(all_in),
                out_names=tuple(out_names), lowering_input_output_aliases=(),
                sim_require_finite=True, sim_require_nnan=True, nc=nc)
            return tuple(outs)

        devices = jax.devices()[:n_cores]
        self.mesh = Mesh(np.asarray(devices), ("core",))
        in_specs = (PartitionSpec("core"),) * (n_params + n_outs)
        out_specs = (PartitionSpec("core"),) * len(out_names)
        self.fn = jax.jit(
            shard_map(_body, mesh=self.mesh, in_specs=in_specs,
                      out_specs=out_specs, check_rep=False),
            keep_unused=True)
        self.sharding = jax.sharding.NamedSharding(self.mesh, PartitionSpec("core"))

    def prepare(self, in_maps):
        n = self.n_cores
        concat_in = [np.concatenate([np.asarray(in_maps[c][nm]) for c in range(n)], axis=0)
                     for nm in self.in_names]
        concat_zero = [np.zeros((n * z.shape[0], *z.shape[1:]), z.dtype)
                       for z in self.zero_outs]
        args = [self.jax.device_put(a, self.sharding) for a in concat_in + concat_zero]
        self.jax.block_until_ready(args)
        return args

    def run(self, args):
        outs = self.fn(*args)
        self.jax.block_until_ready(outs)
        return outs

    def results(self, outs):
        n = self.n_cores
        return [{nm: np.asarray(outs[i]).reshape(n, *self.out_avals[i].shape)[c]
                 for i, nm in enumerate(self.out_names)}
                for c in range(n)]


# ------------------------------------------------------------- entry point

_CACHE = {}


def _rot6d_to_matrix(d6):
    a1, a2 = d6[..., :3], d6[..., 3:]
    b1 = a1 / np.linalg.norm(a1, axis=-1, keepdims=True)
    b2 = a2 - np.sum(b1 * a2, axis=-1, keepdims=True) * b1
    b2 = b2 / np.linalg.norm(b2, axis=-1, keepdims=True)
    b3 = np.cross(b1, b2)
    return np.stack([b1, b2, b3], axis=-2)


def _finish(ne_full, host, params, dt):
    seg, counts = host['seg'], host['counts']
    nodes_n, n_inst = host['nodes_n'], host['n_inst']
    std = POSVEL_STD
    sums = np.zeros((n_inst, HID), np.float32)
    np.add.at(sums, seg, ne_full)
    eff_mean = sums / counts[:, None]
    pm = [(np.asarray(W), np.asarray(b)) for W, b in params['node_out']]
    pred = _mlp_np(pm, eff_mean) * np.float32(dt)
    t = pred[:, :3] * std[:3]
    R = _rot6d_to_matrix(pred[:, 3:])
    p0 = nodes_n[:, :3] * std[:3]
    cs = np.zeros((n_inst, 3), np.float32)
    np.add.at(cs, seg, p0)
    c = cs / counts[:, None]
    p1 = np.einsum('ni,nij->nj', p0 - c[seg], R[seg]) + t[seg] + c[seg]
    return ((p1 - p0) / np.float32(dt)).astype(np.float32)


def kernel(nodes, node_attrs, rels, rel_attrs, rel_stages, instance_idx,
           prop_steps, dt, params):
    assert int(prop_steps) == PROP_STEPS
    in_maps, cfg, host = _prep(np.asarray(nodes), np.asarray(node_attrs),
                               np.asarray(rels), np.asarray(rel_attrs),
                               np.asarray(rel_stages), np.asarray(instance_idx),
                               params)
    key = (tuple(cfg['L'][0]), tuple(cfg['L'][1]))
    if key not in _CACHE:
        nc = build_nc(cfg)
        runner = _Runner(nc, N_CORES)
        _CACHE[key] = runner
    runner = _CACHE[key]
    args = runner.prepare(in_maps)
    outs = runner.run(args)
    res = runner.results(outs)
    ne_full = np.concatenate([res[c]['ne_out'][:SLICE] for c in range(N_CORES)], axis=0)
    return _finish(ne_full, host, params, float(dt))
